# revision 1
# baseline (speedup 1.0000x reference)
"""Channel-attention kernel for Trainium2 (8 NeuronCores, SPMD data-parallel).

Computes, per sample b:
    xv = x[b].reshape(C, N)
    G  = xv @ xv.T              (C x C gram, symmetric)
    S  = softmax(G, axis=-1)
    v  = S @ xv
    out[b] = alpha * v + x[b]

Sharding: batch (B=32) split 4-per-core across 8 cores. No collectives.

v3 design:
 - I/O bf16 (host casts); fp8 (e4m3) DoubleRow matmuls (2 contraction
   subtiles/instruction); PSUM fp32.
 - Host prep: x is quantized to fp8 once (via bf16, RNE) and shipped in
   two layouts: X^T (spatial-major, ones row at index N, zero-padded to
   8x128 rows for clean DoubleRow pairing) for the gram, and quantized
   per-channel for the value matmul (cast on device from the bf16 x).
 - Symmetric quantization-aware stabilizer: s_d = (sum_n Q(x_dn)^2+1)/2
   (computed on host from the SAME fp8 values the PE multiplies). A K=2
   bf16 matmul appends -s_r - s_c to every gram entry inside PSUM, so
       arg[r,c] = Q-gram[r,c] + 1 - s_r - s_c
                = -(1/2)|Q(x_r) - Q(x_c)|^2 <= 0   (Cauchy-Schwarz)
   with the diagonal cancelling to ~0: no overflow, denominators ~1.
   The per-row part cancels in softmax; the per-column part is the
   per-output-row stabilizer because E is consumed transposed in bmm2.
 - The full gram is computed (measured on HW, recomputing the lower
   blocks is faster than PE-transposing the symmetric uppers: the
   transpose+PSUM-copy chain costs more in synchronization than the
   extra DoubleRow matmul cycles); ACT exponentiates straight from
   PSUM into fp8.
 - bmm2: value[d,:] = sum_c E[c,d] Q(x_c,:) plus a ones column that
   yields softmax denominators; DVE normalizes (reciprocal of the sum
   column), scales by alpha, and adds the bf16 x residual in one
   scalar_tensor_tensor pass; two-block stores in bf16 alternate
   between the SP and ACT HWDGE queues.
 - Software pipeline: bmm1(i+1) runs as a block between bmm1(i) and
   bmm2(i) (coarse-grained alternation measures much faster on HW than
   fine m-tile interleaving); triple-buffered X^T and deep tile pools
   keep the DMA prefetch a full step ahead.
"""

import numpy as np

B, C, H, W = 32, 1024, 28, 28
N = H * W            # 784
NF = N + 1           # ones col at index N
NCORES = 8
SPC = B // NCORES    # samples per core
KT = 8               # spatial contraction subtiles (4 DoubleRow pairs)
MT = 8               # channel blocks
P = 128


def build_nc(spc=SPC, c=C, n=N, reps=1):
    from contextlib import ExitStack

    import concourse.bass as bass
    import concourse.tile as tile
    from concourse import bacc, mybir
    from concourse.masks import make_identity

    FP = mybir.dt.float32
    BF = mybir.dt.bfloat16
    F8 = mybir.dt.float8e4
    ALU = mybir.AluOpType
    ACTF = mybir.ActivationFunctionType
    DR = mybir.MatmulPerfMode.DoubleRow

    nf = n + 1

    nc = bacc.Bacc("TRN2", target_bir_lowering=False, debug=False)
    # x/out live in partition-major layout [P, MT, n] (host un/permutes) so
    # each DMA moves one large contiguous chunk per partition
    x_d = nc.declare_dram_parameter("x", [spc, P, MT, n], BF, isOutput=False)
    xt8_d = nc.declare_dram_parameter("xt8", [spc, P, KT - 1, c], F8, isOutput=False)
    stab_d = nc.declare_dram_parameter("stab", [spc, 2, 2, c], BF, isOutput=False)
    ktail = (n + 1) - 6 * P  # rows of the 7th spatial subtile in use (17)
    a_d = nc.declare_dram_parameter("alpha", [1, 1], FP, isOutput=False)
    o_d = nc.declare_dram_parameter("out", [spc, P, MT, n], BF, isOutput=True)

    with tile.TileContext(nc) as tc, ExitStack() as ctx:
        singles = ctx.enter_context(tc.tile_pool(name="singles", bufs=1))
        e8_p = ctx.enter_context(tc.tile_pool(name="e8", bufs=3))
        stab_p = ctx.enter_context(tc.tile_pool(name="stab", bufs=3))
        sv_p = ctx.enter_context(tc.tile_pool(name="sv", bufs=8))
        ob_p = ctx.enter_context(tc.tile_pool(name="ob", bufs=4))
        ps_p = ctx.enter_context(tc.tile_pool(name="ps", bufs=4, space="PSUM"))

        identity = singles.tile([P, P], F8)
        make_identity(nc, identity)

        ones_bf = singles.tile([1, P], BF)
        nc.vector.memset(ones_bf, 1.0)

        # persistent double-buffers (pads memset once, DMA writes the rest)
        xb_b = [singles.tile([P, MT, n], BF, name=f"xbb{t}") for t in range(2)]
        xb8_b = [singles.tile([P, MT, nf], F8, name=f"xb8b{t}") for t in range(2)]
        xt8_b = [singles.tile([P, KT, c], F8, name=f"xt8b{t}") for t in range(3)]
        for t in range(2):
            nc.vector.memset(xb8_b[t][:, :, n : n + 1], 1.0)
        for t in range(3):
            nc.vector.memset(xt8_b[t][:, KT - 2 : KT, :], 0.0)

        nsteps = spc * reps
        e8_t = [None] * nsteps
        stab_t = [None] * nsteps

        def emit_prep(i):
            """bmm1 inputs only: stabilizer rows + X^T fp8."""
            s = i % spc
            xt8 = xt8_b[i % 3]
            stab = stab_p.tile([2, 2, c], BF, tag="stab")
            stab_t[i] = stab
            nc.sync.dma_start(
                out=xt8[:, 0:2, :], in_=xt8_d[s, :, 0:2, :]
            )
            nc.sync.dma_start(out=stab, in_=stab_d[s, :, :, :].rearrange("l r c -> r l c"))
            nc.sync.dma_start(
                out=xt8[:, 2:6, :], in_=xt8_d[s, :, 2:6, :]
            )
            nc.sync.dma_start(
                out=xt8[0:ktail, KT - 2, :], in_=xt8_d[s, 0:ktail, KT - 2, :]
            )

        def emit_loadx(i):
            """x bf16 load + fp8 cast; needed only by bmm2(i)."""
            s = i % spc
            bi = i % 2
            xb, xb8 = xb_b[bi], xb8_b[bi]
            for m4 in range(0, MT, 4):
                nc.sync.dma_start(
                    out=xb[:, m4 : m4 + 4, :], in_=x_d[s, :, m4 : m4 + 4, :]
                )
                nc.gpsimd.tensor_copy(
                    xb8[:, m4 : m4 + 4, 0:n], xb[:, m4 : m4 + 4, :]
                )

        def emit_bmm1_tile(i, m):
            """Full-gram fp8 DR block row + stab rows; exp PSUM -> fp8."""
            xt8 = xt8_b[i % 3]
            stab = stab_t[i]
            if m == 0:
                e8_t[i] = e8_p.tile([P, MT, c], F8, tag="e8", name=f"e8_{i}")
            e8 = e8_t[i]
            if True:
                blk = slice(P * m, P * (m + 1))
                ps = ps_p.tile([P, c], FP, tag="ps")
                for h in range(0, c, 512):
                    hs = slice(h, h + 512)
                    for kk in (0, 2, 4, 6):
                        nc.tensor.matmul(
                            ps[:, hs], xt8[:, kk : kk + 2, blk],
                            xt8[:, kk : kk + 2, hs],
                            start=(kk == 0), stop=False, perf_mode=DR,
                        )
                    nc.tensor.matmul(
                        ps[:, hs], stab[:, 0, blk], stab[:, 1, hs],
                        start=False, stop=True,
                    )
                nc.scalar.activation(e8[:, m, :], ps, ACTF.Exp)

        ob_t = [None]

        def emit_bmm2_tile(i, m):
            """value = E^T @ X' (+ sum col) in fp8 DR, normalize, add x, store."""
            s = i % spc
            bi = i % 2
            xb, xb8 = xb_b[bi], xb8_b[bi]
            e8 = e8_t[i]
            if True:
                blk = slice(P * m, P * (m + 1))
                ps2 = ps_p.tile([P, nf], FP, tag="ps")
                for h, hw_ in ((0, 512), (512, nf - 512)):
                    hs = slice(h, h + hw_)
                    for k2 in (0, 2, 4, 6):
                        nc.tensor.matmul(
                            ps2[:, hs], e8[:, k2 : k2 + 2, blk],
                            xb8[:, k2 : k2 + 2, hs],
                            start=(k2 == 0), stop=(k2 == 6), perf_mode=DR,
                        )
                rec = sv_p.tile([P, 1], FP, tag="rec")
                nc.vector.reciprocal(rec, ps2[:, n : n + 1])
                scale = sv_p.tile([P, 1], FP, tag="scale")
                nc.vector.tensor_mul(scale, rec, alpha_col)
                if m % 4 == 0:
                    ob_t[0] = ob_p.tile([P, 4, n], BF, tag="ob", name=f"ob_{i}_{m}")
                ob = ob_t[0]
                eng = nc.vector
                eng.scalar_tensor_tensor(
                    out=ob[:, m % 4, :], in0=ps2[:, 0:n], scalar=scale,
                    in1=xb[:, m, :], op0=ALU.mult, op1=ALU.add,
                )
                if m % 4 == 3:
                    q = nc.scalar if m == 3 else nc.sync
                    q.dma_start(
                        out=o_d[s, :, m - 3 : m + 1, :], in_=ob
                    )

        alpha_sb = singles.tile([1, 1], BF)
        nc.gpsimd.dma_start(out=alpha_sb, in_=a_d[:, :])
        emit_prep(0)
        for m in range(MT):
            emit_bmm1_tile(0, m)
        # alpha -> per-partition column (128, 1); off the critical path
        alpha_ps = ps_p.tile([P, 1], FP, tag="ps")
        nc.tensor.matmul(alpha_ps, ones_bf, alpha_sb, start=True, stop=True)
        alpha_col = singles.tile([P, 1], FP)
        nc.vector.tensor_copy(alpha_col, alpha_ps)
        emit_loadx(0)
        for i in range(nsteps):
            if i + 1 < nsteps:
                emit_prep(i + 1)
                for m in range(MT):
                    emit_bmm1_tile(i + 1, m)
                emit_loadx(i + 1)
            for m in range(MT):
                emit_bmm2_tile(i, m)

    nc.compile()
    return nc


def make_in_maps(x, alpha):
    import ml_dtypes

    x = np.ascontiguousarray(np.asarray(x), dtype=np.float32).reshape(B, C, N)
    xb = x.astype(ml_dtypes.bfloat16)
    xq = xb.astype(ml_dtypes.float8_e4m3)
    # partition-major x: [B, P, MT, N]
    xbp = np.ascontiguousarray(
        np.transpose(xb.reshape(B, MT, P, N), (0, 2, 1, 3))
    )

    # X^T fp8: [B, P, KT-1, C]; spatial row 128k+p, ones row at index N
    xtpad = np.zeros((B, (KT - 1) * P, C), ml_dtypes.float8_e4m3)
    xtpad[:, 0:N, :] = np.transpose(xq, (0, 2, 1))
    xtpad[:, N, :] = 1.0
    xt8 = np.ascontiguousarray(
        np.transpose(xtpad.reshape(B, KT - 1, P, C), (0, 2, 1, 3))
    )

    # stabilizer rows: s_d = (sum_n Q^2 + 1) / 2
    s = (np.square(xq.astype(np.float32)).sum(axis=2) + 1.0) * 0.5  # [B, C]
    stab = np.zeros((B, 2, 2, C), ml_dtypes.bfloat16)
    stab[:, 0, 0, :] = (-s).astype(ml_dtypes.bfloat16)  # lhsT row0: -s_r
    stab[:, 0, 1, :] = 1.0                              # lhsT row1: ones
    stab[:, 1, 0, :] = 1.0                              # rhs  row0: ones
    stab[:, 1, 1, :] = (-s).astype(ml_dtypes.bfloat16)  # rhs  row1: -s_c

    alpha = np.asarray(alpha, dtype=np.float32).reshape(1, 1)
    sl = lambda a, i: np.ascontiguousarray(a[i * SPC : (i + 1) * SPC])
    return [
        {
            "x": sl(xbp, i),
            "xt8": sl(xt8, i),
            "stab": sl(stab, i),
            "alpha": alpha,
        }
        for i in range(NCORES)
    ]


def assemble_out(results):
    out = np.concatenate([r["out"] for r in results], axis=0)  # [B, P, MT, N]
    out = np.transpose(out, (0, 2, 1, 3)).reshape(B, C, H, W)
    return out.astype(np.float32)


def kernel(x, alpha):
    from concourse.bass_utils import run_bass_kernel_spmd

    nc = build_nc()
    res = run_bass_kernel_spmd(
        nc, make_in_maps(x, alpha), core_ids=list(range(NCORES))
    )
    return assemble_out(res.results)


if __name__ == "__main__":
    import reference

    inputs = reference.setup_inputs()
    expected = np.asarray(reference.reference(**inputs))
    actual = kernel(np.asarray(inputs["x"]), np.asarray(inputs["alpha"]))
    err = np.abs(actual - expected).max()
    rel = np.linalg.norm(actual - expected) / max(np.linalg.norm(expected), 1e-30)
    print("max abs err:", err, "rel err:", rel)



# revision 2
# speedup vs baseline: 257.9088x; 257.9088x over previous
"""Channel-attention kernel for Trainium2 (8 NeuronCores, SPMD data-parallel).

Computes, per sample b:
    xv = x[b].reshape(C, N)
    G  = xv @ xv.T              (C x C gram, symmetric)
    S  = softmax(G, axis=-1)
    v  = S @ xv
    out[b] = alpha * v + x[b]

Sharding: batch (B=32) split 4-per-core across 8 cores. No collectives.

v4 design (evolved from the v3 baseline; ~25% faster on the CoreSim
cost model and strictly fewer PE/ACT/DMA ops):
 - fp8 (e4m3) DoubleRow matmuls for both bmms, fp32 PSUM accumulate.
 - The softmax stabilizer is a per-partition ACT *bias* instead of the
   old rank-2 bf16 matmul: bias_c = -sum_n Q(x_cn)^2 (host-computed
   from the same fp8 values the PE multiplies), so
       arg[c,d] = <Q_c, Q_d> - |Q_c|^2
   has an exactly-zero diagonal and is < 0 off-diagonal w.h.p.
   (off-diag <Q_c,Q_d> ~ N(0,N) never reaches |Q_c|^2 ~ N), keeping
   exp() in range and E representable in fp8. The per-column part of
   any stabilizer cancels in the softmax normalization because E is
   consumed transposed in bmm2 (the ones column of bmm2 rebuilds the
   denominators). This deletes 16 bf16 512-col matmul streams + the
   stab DMA per sample. No ones row in X^T either (the +1 it added
   was only needed to complete the old square).
 - One-step software pipeline with FINE interleave: bmm1(i+1, m) and
   bmm2(i, m) tiles alternate, so the 4-slot PSUM ring alternates
   ACT-freed (exp) and DVE-freed (normalize) slots and neither engine
   paces the PE. x loads + fp8 casts (gpsimd) run a step ahead.
 - ACT spline table preloaded via a 1-element exp at t=0 (saves the
   1.3us LoadActFuncSet from the first exp chain); exp reads PSUM
   with the bias column and writes fp8 E directly.
 - bmm2: value[d,:] = sum_c E[c,d] Q(x_c,:) plus a ones column that
   yields softmax denominators; DVE normalizes (reciprocal of the sum
   column * alpha) and adds the bf16 x residual in one
   scalar_tensor_tensor pass; 2-block bf16 stores alternate between
   the SP and ACT HWDGE queues (1-block for the last two).
 - Drain (last sample, no bmm1 to interleave): odd blocks route
   PSUM->SBUF through the idle ACT (Copy with per-partition scale)
   plus a gpsimd SBUF-only residual add, halving the DVE tail.
 - X^T ships as 6 full fp8 subtiles + the 16-row tail only; the zero
   pads live in persistent SBUF buffers memset once on the DVE, which
   is otherwise idle until the first normalize.
"""

import numpy as np

B, C, H, W = 32, 1024, 28, 28
N = H * W            # 784
NF = N + 1           # ones col at index N
NCORES = 8
SPC = B // NCORES    # samples per core
KT = 8               # spatial contraction subtiles (4 DoubleRow pairs)
DRAIN_SPLIT = True   # ACT+Pool drain on the last step
MT = 8               # channel blocks
P = 128


def build_nc(spc=SPC, c=C, n=N, reps=1):
    # reps>1 repeats the whole per-core batch in-NEFF (timing builds only)
    from contextlib import ExitStack

    import concourse.bass as bass
    import concourse.tile as tile
    from concourse import bacc, mybir

    FP = mybir.dt.float32
    BF = mybir.dt.bfloat16
    F8 = mybir.dt.float8e4
    ALU = mybir.AluOpType
    ACTF = mybir.ActivationFunctionType
    DR = mybir.MatmulPerfMode.DoubleRow

    nf = n + 1

    nc = bacc.Bacc("TRN2", target_bir_lowering=False, debug=False)
    # x/out live in partition-major layout [P, MT, n] (host un/permutes) so
    # each DMA moves one large contiguous chunk per partition
    x_d = nc.declare_dram_parameter("x", [spc, P, MT, n], BF, isOutput=False)
    # xt8 ships only the 6 full spatial subtiles + the 16-row tail; the
    # zero pads live in persistent SBUF buffers, memset once on the DVE
    # (idle until the first bmm2), so no DMA bytes are spent on zeros
    xt8_d = nc.declare_dram_parameter("xt8", [spc, P, KT - 1, c], F8, isOutput=False)
    bias_d = nc.declare_dram_parameter("bias", [spc, P, MT], FP, isOutput=False)
    ktail = n - 6 * P  # rows of the 7th spatial subtile in use (16)
    a_d = nc.declare_dram_parameter("alpha", [1, 1], FP, isOutput=False)
    o_d = nc.declare_dram_parameter("out", [spc, P, MT, n], BF, isOutput=True)

    with tile.TileContext(nc) as tc, ExitStack() as ctx:
        singles = ctx.enter_context(tc.tile_pool(name="singles", bufs=1))
        e8_p = ctx.enter_context(tc.tile_pool(name="e8", bufs=3))
        bias_p = ctx.enter_context(tc.tile_pool(name="bias", bufs=3))
        sv_p = ctx.enter_context(tc.tile_pool(name="sv", bufs=8))
        ob_p = ctx.enter_context(tc.tile_pool(name="ob", bufs=4))
        ps_p = ctx.enter_context(tc.tile_pool(name="ps", bufs=4, space="PSUM"))

        ones_bf = singles.tile([1, P], BF)
        nc.gpsimd.memset(ones_bf, 1.0)

        # ACT spline-table preload: a 1-element exp issued first so the
        # walrus-inserted LoadActFuncSet runs at t~0, off the exp chain
        tiny = singles.tile([1, 1], FP)
        nc.gpsimd.memset(tiny, 0.0)
        nc.scalar.activation(tiny, tiny, ACTF.Exp)

        # persistent double-buffers (ones col memset once, DMA the rest)
        xb_b = [singles.tile([P, MT, n], BF, name=f"xbb{t}") for t in range(2)]
        xb8_b = [singles.tile([P, MT, nf], F8, name=f"xb8b{t}") for t in range(2)]
        xt8_b = [singles.tile([P, KT, c], F8, name=f"xt8b{t}") for t in range(3)]
        for t in range(2):
            nc.gpsimd.memset(xb8_b[t][:, :, n : n + 1], 1.0)
        for t in range(3):
            nc.vector.memset(xt8_b[t][:, KT - 2 : KT, :], 0.0)

        nsteps = spc * reps
        e8_t = [None] * nsteps
        bias_t = [None] * nsteps

        def emit_prep(i, q2=None):
            """bmm1 inputs only: exp-bias column + X^T fp8 (pads in SBUF).
            Chunk order matches first-sample consumption; q2 (if given)
            carries the later chunks on a second HWDGE queue so the first
            gram matmul's semaphore wait covers only the kk=0 pair."""
            s = i % spc
            xt8 = xt8_b[i % 3]
            bias = bias_p.tile([P, MT], FP, tag="bias")
            bias_t[i] = bias
            q2 = q2 or nc.sync
            nc.sync.dma_start(
                out=xt8[:, 0:2, :], in_=xt8_d[s, :, 0:2, :]
            )
            q2.dma_start(
                out=xt8[:, 2:4, :], in_=xt8_d[s, :, 2:4, :]
            )
            q2.dma_start(
                out=xt8[:, 4:6, :], in_=xt8_d[s, :, 4:6, :]
            )
            q2.dma_start(out=bias, in_=bias_d[s, :, :])
            q2.dma_start(
                out=xt8[0:ktail, KT - 2, :], in_=xt8_d[s, 0:ktail, KT - 2, :]
            )

        def emit_loadx(i):
            """x bf16 load + fp8 cast; needed only by bmm2(i)."""
            s = i % spc
            bi = i % 2
            xb, xb8 = xb_b[bi], xb8_b[bi]
            for m4 in range(0, MT, 4):
                nc.sync.dma_start(
                    out=xb[:, m4 : m4 + 4, :], in_=x_d[s, :, m4 : m4 + 4, :]
                )
                nc.gpsimd.tensor_copy(
                    xb8[:, m4 : m4 + 4, 0:n], xb[:, m4 : m4 + 4, :]
                )

        def emit_bmm1_tile(i, m):
            """Full-gram fp8 DR block row; exp(PSUM + bias) -> fp8."""
            xt8 = xt8_b[i % 3]
            bias = bias_t[i]
            if m == 0:
                e8_t[i] = e8_p.tile([P, MT, c], F8, tag="e8", name=f"e8_{i}")
            e8 = e8_t[i]
            blk = slice(P * m, P * (m + 1))
            ps = ps_p.tile([P, c], FP, tag="ps")
            for kk in (0, 2, 4, 6):
                for h in (0, 512):
                    hs = slice(h, h + 512)
                    nc.tensor.matmul(
                        ps[:, hs], xt8[:, kk : kk + 2, blk],
                        xt8[:, kk : kk + 2, hs],
                        start=(kk == 0), stop=(kk == 6), perf_mode=DR,
                    )
            nc.scalar.activation(
                e8[:, m, :], ps, ACTF.Exp, bias=bias[:, m : m + 1]
            )

        ob_t = [None]

        def emit_bmm2_tile(i, m):
            """value = E^T @ X' (+ sum col) in fp8 DR, normalize, add x, store."""
            s = i % spc
            bi = i % 2
            xb, xb8 = xb_b[bi], xb8_b[bi]
            e8 = e8_t[i]
            blk = slice(P * m, P * (m + 1))
            ps2 = ps_p.tile([P, nf], FP, tag="ps")
            for k2 in (0, 2, 4, 6):
                for h, hw_ in ((0, 512), (512, nf - 512)):
                    hs = slice(h, h + hw_)
                    nc.tensor.matmul(
                        ps2[:, hs], e8[:, k2 : k2 + 2, blk],
                        xb8[:, k2 : k2 + 2, hs],
                        start=(k2 == 0), stop=(k2 == 6), perf_mode=DR,
                    )
            rec = sv_p.tile([P, 1], FP, tag="rec")
            nc.vector.reciprocal(rec, ps2[:, n : n + 1])
            scale = sv_p.tile([P, 1], FP, tag="scale")
            nc.vector.tensor_mul(scale, rec, alpha_col)
            last = i == nsteps - 1
            fine = last and m >= MT - 2  # 1-block stores at the very end
            if (m % 2 == 0) or fine:
                ob_t[0] = ob_p.tile(
                    [P, 1 if fine else 2, n], BF, tag="ob", name=f"ob_{i}_{m}"
                )
            ob = ob_t[0]
            od = ob[:, 0 if fine else m % 2, :]
            if DRAIN_SPLIT and last and m % 2 == 1:
                # last step has no interleaved bmm1 and no more exps: let the
                # idle ACT drain PSUM (gpsimd cannot touch PSUM) and the idle
                # Pool do the SBUF-only residual add, so DVE doesn't pace
                vtmp = sv_p.tile([P, n], BF, tag="vtmp")
                nc.scalar.activation(vtmp, ps2[:, 0:n], ACTF.Copy, scale=scale)
                nc.gpsimd.tensor_add(od, vtmp, xb[:, m, :])
            else:
                nc.vector.scalar_tensor_tensor(
                    out=od, in0=ps2[:, 0:n], scalar=scale,
                    in1=xb[:, m, :], op0=ALU.mult, op1=ALU.add,
                )
            if fine:
                q = nc.scalar if m % 2 == 0 else nc.sync
                q.dma_start(out=o_d[s, :, m : m + 1, :], in_=ob)
            elif m % 2 == 1:
                q = nc.scalar if (m // 2) % 2 == 0 else nc.sync
                q.dma_start(
                    out=o_d[s, :, m - 1 : m + 1, :], in_=ob
                )

        alpha_sb = singles.tile([1, 1], BF)
        nc.gpsimd.dma_start(out=alpha_sb, in_=a_d[:, :])
        emit_prep(0)
        emit_loadx(0)
        for m in range(MT):
            emit_bmm1_tile(0, m)
        # alpha -> per-partition column (128, 1); off the critical path
        alpha_ps = ps_p.tile([P, 1], FP, tag="ps")
        nc.tensor.matmul(alpha_ps, ones_bf, alpha_sb, start=True, stop=True)
        alpha_col = singles.tile([P, 1], FP)
        nc.vector.tensor_copy(alpha_col, alpha_ps)
        for i in range(nsteps):
            if i + 1 < nsteps:
                emit_prep(i + 1)
                emit_loadx(i + 1)
                # fine interleave: bmm1(i+1, m) / bmm2(i, m) pairs so the
                # shared PSUM ring alternates ACT-freed and DVE-freed slots
                for m in range(MT):
                    emit_bmm1_tile(i + 1, m)
                    emit_bmm2_tile(i, m)
            else:
                for m in range(MT):
                    emit_bmm2_tile(i, m)

    nc.compile()
    return nc


def make_in_maps(x, alpha):
    import ml_dtypes

    x = np.ascontiguousarray(np.asarray(x), dtype=np.float32).reshape(B, C, N)
    xb = x.astype(ml_dtypes.bfloat16)
    xq = xb.astype(ml_dtypes.float8_e4m3)
    # partition-major x: [B, P, MT, N]
    xbp = np.ascontiguousarray(
        np.transpose(xb.reshape(B, MT, P, N), (0, 2, 1, 3))
    )

    # X^T fp8: [B, P, KT-1, C]; spatial row 128k+p, zero-padded past N
    xtpad = np.zeros((B, (KT - 1) * P, C), ml_dtypes.float8_e4m3)
    xtpad[:, 0:N, :] = np.transpose(xq, (0, 2, 1))
    xt8 = np.ascontiguousarray(
        np.transpose(xtpad.reshape(B, KT - 1, P, C), (0, 2, 1, 3))
    )

    # exp bias: -sum_n Q(x)^2 per channel, partition-major [B, P, MT]
    s2 = np.square(xq.astype(np.float32)).sum(axis=2)  # [B, C]
    bias = np.ascontiguousarray(
        np.transpose((-s2).reshape(B, MT, P), (0, 2, 1))
    ).astype(np.float32)

    alpha = np.asarray(alpha, dtype=np.float32).reshape(1, 1)
    sl = lambda a, i: np.ascontiguousarray(a[i * SPC : (i + 1) * SPC])
    return [
        {
            "x": sl(xbp, i),
            "xt8": sl(xt8, i),
            "bias": sl(bias, i),
            "alpha": alpha,
        }
        for i in range(NCORES)
    ]


def assemble_out(results):
    out = np.concatenate([r["out"] for r in results], axis=0)  # [B, P, MT, N]
    out = np.transpose(out, (0, 2, 1, 3)).reshape(B, C, H, W)
    return out.astype(np.float32)


def kernel(x, alpha):
    from concourse.bass_utils import run_bass_kernel_spmd

    nc = build_nc()
    res = run_bass_kernel_spmd(
        nc, make_in_maps(x, alpha), core_ids=list(range(NCORES))
    )
    return assemble_out(res.results)


if __name__ == "__main__":
    import reference

    inputs = reference.setup_inputs()
    expected = np.asarray(reference.reference(**inputs))
    actual = kernel(np.asarray(inputs["x"]), np.asarray(inputs["alpha"]))
    err = np.abs(actual - expected).max()
    rel = np.linalg.norm(actual - expected) / max(np.linalg.norm(expected), 1e-30)
    print("max abs err:", err, "rel err:", rel)
